# revision 3
# baseline (speedup 1.0000x reference)
"""Trainium2 Bass kernel for nn_Diff_Label01_Loss — v7 (split AR + hybrid pass-2 per half).

Single ncfw all-reduce (its ~65us-from-start warmup is the hard floor in
this environment; a hand-rolled remote-DMA exchange crashes the virtualized
runtime when HWDGE traffic coexists), with everything else restructured
around fused ops so the tail after the all-reduce is minimal:

  ingest  8 per-tile DMAs [128,4096] bf16, split over 2 DGE rings
  PE      s0 accumulation into psum[1,4096] f32 (mask column stationary)
  ACT     pass-1: fused Square+accum per tile -> nsq[:, t]  (no DVE muls,
          no separate reduce pass); then one psum->bf16 cast; AR input DMA
  CC      AllReduce bf16 8KB (gpsimd) -> cc_out
  bcast   cc_out -> mb[128,4096] via 2 stride-0 DMAs (both rings);
          cc_out -> mpm32 [128,32] for a cheap all-lane msq Square+accum
  DVE     pass-2: fused scalar_tensor_tensor (x*mb)+accum -> num[:, t]
  out     [128,17] f32: num(8) | nsq(8) | msqp(1); host epilogue
"""

import contextlib

import numpy as np

B = 8192
D = 4096
P = 128
NCORES = 8
ROWS = B // NCORES
T = ROWS // P               # 8 row-tiles
MM = 512
EPS = 1e-8


def _build_program():
    import concourse.bass as bass
    import concourse.mybir as mybir

    f32 = mybir.dt.float32
    bf16 = mybir.dt.bfloat16
    AOP = mybir.AluOpType
    AF = mybir.ActivationFunctionType

    nc = bass.Bass(trn_type="TRN2", num_devices=NCORES)

    xb = nc.dram_tensor("xb", [ROWS, D], bf16, kind="ExternalInput")
    m0b = nc.dram_tensor("m0b", [P, T], bf16, kind="ExternalInput")
    out = nc.dram_tensor("out", [P, 25], f32, kind="ExternalOutput")
    H = D // 2
    cc_in_a = nc.dram_tensor("cc_in_a", [P, 16], bf16)
    cc_out_a = nc.dram_tensor("cc_out_a", [P, 16], bf16, addr_space="Shared")
    cc_in_b = nc.dram_tensor("cc_in_b", [P, 16], bf16)
    cc_out_b = nc.dram_tensor("cc_out_b", [P, 16], bf16, addr_space="Shared")

    ctx = contextlib.ExitStack()

    def sb(name, shape, dt):
        return ctx.enter_context(nc.sbuf_tensor(name, shape, dt))

    x_all = sb("x_all", [P, T * D], bf16)
    m0s = sb("m0s", [P, T], bf16)
    s0bf = sb("s0bf", [1, D], bf16)
    mb = sb("mb", [P, D], bf16)
    mpm32 = sb("mpm32", [P, 32], bf16)
    nsq = sb("nsq", [P, T], f32)
    num = sb("num", [P, 2 * T], f32)
    d1 = sb("d1", [P, D], bf16)
    d2 = sb("d2", [P, D], bf16)
    tmp = [sb(f"tmp{i}", [P, D], bf16) for i in range(3)]
    dq = sb("dq", [P, 32], bf16)
    msqp = sb("msqp", [P, 1], f32)

    psum_s0 = ctx.enter_context(nc.psum_tensor("psum_s0", [1, D], f32))

    def sem(name):
        return ctx.enter_context(nc.semaphore(name))

    dx = [sem(f"dx{t}") for t in range(T)]
    sm0 = sem("sm0")
    s_pe = sem("s_pe")
    s_cast = sem("s_cast")
    s_ccin = sem("s_ccin")
    s_cc = sem("s_cc")      # +1 per half
    s_mba = sem("s_mba")    # half-A bcast quarters
    s_mbb = sem("s_mbb")
    s_mpm = sem("s_mpm")
    s_p2 = sem("s_p2")
    s_m2 = sem("s_m2")
    s_r2 = sem("s_r2")
    s_sq = sem("s_sq")
    s_msq = sem("s_msq")
    s_outd = sem("s_outd")

    xr = xb.rearrange("(t p) d -> t p d", p=P)

    def x_t(t):
        return x_all[:, t * D:(t + 1) * D]

    with nc.Block() as block:

        @block.sync
        def _(sync):
            for t in range(0, T, 2):
                sync.dma_start(x_t(t), xr[t]).then_inc(dx[t], 16)
            # AR inputs as each half's cast lands
            sync.wait_ge(s_cast, 1)
            sync.dma_start(
                cc_in_a.rearrange("(a p) q -> a (p q)", a=1), s0bf[0:1, 0:H]
            ).then_inc(s_ccin, 16)
            sync.wait_ge(s_cast, 2)
            sync.dma_start(
                cc_in_b.rearrange("(a p) q -> a (p q)", a=1), s0bf[0:1, H:D]
            ).then_inc(s_ccin, 16)
            # half-A quarter 1 on this ring
            sync.wait_ge(s_cc, 1)
            sync.dma_start(
                mb[:, 0 : H // 2],
                cc_out_a.rearrange("(a p) q -> a (p q)", a=1)[0:1, 0 : H // 2].to_broadcast((P, H // 2)),
            ).then_inc(s_mba, 16)
            # half-B both quarters on this ring + mpm32 halves
            sync.wait_ge(s_cc, 2)
            sync.dma_start(
                mb[:, H : H + H // 2],
                cc_out_b.rearrange("(a p) q -> a (p q)", a=1)[0:1, 0 : H // 2].to_broadcast((P, H // 2)),
            ).then_inc(s_mbb, 16)
            sync.dma_start(
                mb[:, H + H // 2 : D],
                cc_out_b.rearrange("(a p) q -> a (p q)", a=1)[0:1, H // 2 : H].to_broadcast((P, H // 2)),
            ).then_inc(s_mbb, 16)
            sync.dma_start(mpm32[:, 0:16], cc_out_a[:]).then_inc(s_mpm, 16)
            sync.dma_start(mpm32[:, 16:32], cc_out_b[:]).then_inc(s_mpm, 16)
            # output
            sync.wait_ge(s_p2, 4)
            sync.wait_ge(s_r2, 12)
            sync.dma_start(out[:, 0:16], num[:]).then_inc(s_outd, 16)
            sync.wait_ge(s_sq, T)
            sync.dma_start(out[:, 16:24], nsq[:]).then_inc(s_outd, 16)
            sync.wait_ge(s_msq, 1)
            with nc.allow_non_contiguous_dma(reason="128x4B msq column"):
                sync.dma_start(out[:, 24:25], msqp[:]).then_inc(s_outd, 16)
            sync.wait_ge(s_outd, 48)

        @block.tensor
        def _(pe):
            pe.wait_ge(sm0, 16)
            for t in range(T):
                pe.wait_ge(dx[t], 16)
                for h in range(D // MM):
                    mm = nc.tensor.matmul(
                        psum_s0[0:1, h * MM:(h + 1) * MM],
                        m0s[:, t : t + 1],
                        x_t(t)[:, h * MM:(h + 1) * MM],
                        start=(t == 0),
                        stop=(t == T - 1),
                    )
                    if t == T - 1 and h == D // MM - 1:
                        mm.then_inc(s_pe, 1)

        @block.scalar
        def _(sc):
            sc.dma_start(m0s[:], m0b[:]).then_inc(sm0, 16)
            for t in range(1, T, 2):
                sc.dma_start(x_t(t), xr[t]).then_inc(dx[t], 16)
            # pass-1 fused squares; cast interleaved right after the last tile
            for t in range(T):
                sc.wait_ge(dx[t], 16)
                if t == T - 1:
                    sc.wait_ge(s_pe, 1)
                    sc.copy(s0bf[0:1, 0:H], psum_s0[0:1, 0:H]).then_inc(s_cast, 1)
                    sc.copy(s0bf[0:1, H:D], psum_s0[0:1, H:D]).then_inc(s_cast, 1)
                sc.activation(
                    d1[:], x_t(t), AF.Square,
                    accum_out=nsq[:, t : t + 1],
                ).then_inc(s_sq, 1)
            # half-A quarter 2 on the ACT ring
            sc.wait_ge(s_cc, 1)
            sc.dma_start(
                mb[:, H // 2 : H],
                cc_out_a.rearrange("(a p) q -> a (p q)", a=1)[0:1, H // 2 : H].to_broadcast((P, H // 2)),
            ).then_inc(s_mba, 16)
            # pass-2 hybrid reduces, half A then half B; msq in between
            for t in range(6):
                sc.wait_ge(s_m2, t + 1)
                b = tmp[t % 3][:, 0:H]
                sc.activation(
                    b, b, AF.Copy, accum_out=num[:, 2 * t : 2 * t + 1]
                ).then_inc(s_r2, 1)
            sc.wait_ge(s_mpm, 32)
            sc.activation(dq[:], mpm32[:], AF.Square, accum_out=msqp[:]).then_inc(s_msq, 1)
            for t in range(6):
                sc.wait_ge(s_m2, 7 + t)
                b = tmp[t % 3][:, H:D]
                sc.activation(
                    b, b, AF.Copy, accum_out=num[:, 2 * t + 1 : 2 * t + 2]
                ).then_inc(s_r2, 1)

        @block.vector
        def _(ve):
            # pass-2 per half: 6 plain muls feed ACT reduces; DVE STTs tiles 6,7
            ve.wait_ge(s_mba, 32)
            for t in range(6):
                if t >= 3:
                    ve.wait_ge(s_r2, t - 2)
                nc.vector.tensor_mul(
                    tmp[t % 3][:, 0:H], x_t(t)[:, 0:H], mb[:, 0:H]
                ).then_inc(s_m2, 1)
            for t in range(6, T):
                nc.vector.scalar_tensor_tensor(
                    out=d2[:, 0:H], in0=x_t(t)[:, 0:H], scalar=1.0, in1=mb[:, 0:H],
                    op0=AOP.mult, op1=AOP.mult,
                    accum_out=num[:, 2 * t : 2 * t + 1],
                ).then_inc(s_p2, 1)
            ve.wait_ge(s_mbb, 32)
            for t in range(6):
                if t >= 3:
                    ve.wait_ge(s_r2, 7 + t - 3)
                nc.vector.tensor_mul(
                    tmp[t % 3][:, H:D], x_t(t)[:, H:D], mb[:, H:D]
                ).then_inc(s_m2, 1)
            for t in range(6, T):
                nc.vector.scalar_tensor_tensor(
                    out=d2[:, H:D], in0=x_t(t)[:, H:D], scalar=1.0, in1=mb[:, H:D],
                    op0=AOP.mult, op1=AOP.mult,
                    accum_out=num[:, 2 * t + 1 : 2 * t + 2],
                ).then_inc(s_p2, 1)

        @block.gpsimd
        def _(gp):
            gp.wait_ge(s_ccin, 16)
            gp.collective_compute(
                "AllReduce",
                AOP.add,
                replica_groups=[list(range(NCORES))],
                ins=[cc_in_a[:]],
                outs=[cc_out_a[:]],
            ).then_inc(s_cc, 1)
            gp.wait_ge(s_ccin, 32)
            gp.collective_compute(
                "AllReduce",
                AOP.add,
                replica_groups=[list(range(NCORES))],
                ins=[cc_in_b[:]],
                outs=[cc_out_b[:]],
            ).then_inc(s_cc, 1)

    ctx.close()
    return nc


_PROGRAM = None
LAST_RESULT = None


def _host_inputs(labels, datas):
    import ml_dtypes

    labels = np.asarray(labels, dtype=np.float32)
    datas = np.asarray(datas, dtype=np.float32)

    mask0 = (labels[:, 0] >= labels[:, 1]).astype(np.float32)
    mask1 = np.float32(1.0) - mask0
    n0 = float(mask0.sum())
    n1 = float(mask1.sum())

    xbf = datas.astype(ml_dtypes.bfloat16)

    in_maps = []
    m0s_l = []
    m1s_l = []
    for c in range(NCORES):
        rows = slice(c * ROWS, (c + 1) * ROWS)
        m0c = mask0[rows].reshape(T, P).T.copy()
        m1c = mask1[rows].reshape(T, P).T.copy()
        in_maps.append(
            {
                "xb": np.ascontiguousarray(xbf[rows]),
                "m0b": m0c.astype(ml_dtypes.bfloat16),
            }
        )
        m0s_l.append(m0c.astype(np.float64))
        m1s_l.append(m1c.astype(np.float64))
    return in_maps, n0, n1, m0s_l, m1s_l


def _host_finish(outs, n0, n1, m0s_l, m1s_l):
    ssim = 0.0
    sdif = 0.0
    for c in range(NCORES):
        o = np.asarray(outs[c], dtype=np.float64)
        numer = o[:, 0:16].reshape(P, T, 2).sum(axis=2)
        xnorm = np.maximum(np.sqrt(o[:, 16:24]), EPS)
        q = np.abs(numer) / xnorm
        ssim += (q * m0s_l[c]).sum()
        sdif += (q * m1s_l[c]).sum()
    msq = float(np.asarray(outs[0], dtype=np.float64)[:, 24].sum())

    if n0 > 0.0:
        mnorm = max(np.sqrt(msq), EPS * n0)
        sim = 1.0 - ssim / (n0 * mnorm)
        dif = (sdif / (n1 * mnorm)) if n1 > 0.0 else 0.0
    else:
        sim = 0.0
        dif = 0.0

    sim = np.float32(sim)
    dif = np.float32(dif)
    return (np.float32(sim + dif), sim, dif)


def kernel(labels, datas):
    global _PROGRAM, LAST_RESULT
    from concourse.bass_utils import run_bass_kernel_spmd

    in_maps, n0, n1, m0s_l, m1s_l = _host_inputs(labels, datas)
    if _PROGRAM is None:
        _PROGRAM = _build_program()
    res = run_bass_kernel_spmd(_PROGRAM, in_maps, list(range(NCORES)))
    LAST_RESULT = res
    outs = [res.results[c]["out"] for c in range(NCORES)]
    return _host_finish(outs, n0, n1, m0s_l, m1s_l)
